# revision 13
# baseline (speedup 1.0000x reference)
"""Trainium2 Bass kernel for nn_LoRALinear4bit.

Computes  out = x @ dequant_nf4(q_idx, absmax).T + (x @ A) @ B * 2.0
with x [4,2048,4096] f32, q_idx [4096,4096] int32 (NF4 codes),
absmax [4096,64] f32 (per-64-block scales), A [4096,16], B [16,4096].

Strategy (column / tensor parallel over 8 NeuronCores):
  * shard out_features OUT=4096 into 8 x 512; replicate x, A.
  * per core, on device:
      - dequantize W^T shard [IN, 512] from host-transposed q codes via a
        degree-15 interpolating polynomial of the NF4 codebook (exact at the
        16 integer nodes up to ~6e-6 abs err), times the expanded absmax.
      - fold the LoRA product into the weight: W_eff = W^T*scale + 2*(A@B).
      - one big matmul: out_shard[8192, 512] = x @ W_eff, accumulated over
        IN in 32 K-tiles of 128, fp32r (full-rate fp32 PE mode).
  * host gathers the 8 column shards.

Host-side work is layout only: transposes (x.T, q.T, A.T), shard slicing,
and absmax block expansion (np.repeat).
"""

import numpy as np

# problem shape (hardcoded per contract: kernel.py must be self-contained)
B_, S_, IN, OUT = 4, 2048, 4096, 4096
TOK = B_ * S_            # 8192 tokens
NCORES = 8
OSH = OUT // NCORES      # 512 out-features per core
R = 16                   # LoRA rank
SCALING = 2.0            # alpha/r = 32/16
QBLOCK = 64              # bnb absmax blocksize

KT = IN // 128           # 32 K tiles
TG = 512                 # token group per x DMA
NG = TOK // TG           # 16 token groups
MPG = TG // 128          # 4 m-tiles per group

# bitsandbytes NF4 codebook
NF4 = np.array([
    -1.0, -0.6961928009986877, -0.5250730514526367, -0.39491748809814453,
    -0.28444138169288635, -0.18477343022823334, -0.09105003625154495, 0.0,
    0.07958029955625534, 0.16093020141124725, 0.24611230194568634,
    0.33791524171829224, 0.44070982933044434, 0.5626170039176941,
    0.6989699602127075, 1.0], dtype=np.float64)


def _poly_coeffs():
    """Coefficients of the degree-15 interpolating polynomial of NF4 in
    u = (q - 7.5)/7.5 (monomial basis, increasing order)."""
    q = np.arange(16, dtype=np.float64)
    u = (q - 7.5) / 7.5
    V = np.vander(u, 16, increasing=True)
    return np.linalg.solve(V, NF4)


_CACHE = {}


def _build(mm_dtype_name="float32r"):
    """Build + compile the per-core Bass program (identical on all cores)."""
    key = mm_dtype_name
    if key in _CACHE:
        return _CACHE[key]

    import concourse.bacc as bacc
    import concourse.tile as tile
    from concourse import mybir
    from concourse.bass import ts, ds

    f32 = mybir.dt.float32
    i32 = mybir.dt.int32
    mm_dt = getattr(mybir.dt, mm_dtype_name)
    Alu = mybir.AluOpType

    c = _poly_coeffs()

    nc = bacc.Bacc("TRN2", target_bir_lowering=False, debug=False)

    xt = nc.dram_tensor("xt", [IN, TOK], f32, kind="ExternalInput").ap()
    qt = nc.dram_tensor("qt", [IN, OSH], i32, kind="ExternalInput").ap()
    scl = nc.dram_tensor("scl", [IN, OSH], f32, kind="ExternalInput").ap()
    at = nc.dram_tensor("at", [R, IN], f32, kind="ExternalInput").ap()
    bsh = nc.dram_tensor("bsh", [R, OSH], f32, kind="ExternalInput").ap()
    out = nc.dram_tensor("out", [TOK, OSH], f32, kind="ExternalOutput").ap()

    # token groups whose contraction is split k<KH | k>=KH so their first
    # half can run while dequant is still producing the later weff tiles
    KH = KT // 2
    NG_SPLIT = 8

    with tile.TileContext(nc) as tc:
        with (
            tc.tile_pool(name="weff", bufs=1) as weff_pool,
            tc.tile_pool(name="deq", bufs=2) as deq_pool,
            tc.tile_pool(name="part", bufs=1) as part_pool,
            tc.tile_pool(name="xin", bufs=6) as x_pool,
            tc.tile_pool(name="oup", bufs=3) as o_pool,
            tc.tile_pool(name="wadd_ps", bufs=1, space="PSUM") as wadd_pool,
            tc.tile_pool(name="mm_ps", bufs=6, space="PSUM") as mm_pool,
            tc.tile_pool(name="const", bufs=1) as const_pool,
        ):
            # resident constants
            b_sb = const_pool.tile([R, OSH], f32, tag="b_sb", name="b_sb")
            nc.gpsimd.dma_start(out=b_sb[:], in_=bsh[:])

            # ---- Phase A: W_eff[k] = NF4[q^T]*scale + 2*(A@B)
            # Processed as supertiles of SW k-tiles (wide free dim amortizes
            # the per-op DVE overhead of the 16-step Horner chain).
            SW = 2
            W = SW * OSH
            weff_s = []
            for j in range(KT // SW):
                w = weff_pool.tile([128, W], mm_dt, tag=f"weff{j}",
                                   name=f"weff{j}")
                weff_s.append(w)
            weff = [weff_s[k // SW][:, ts(k % SW, OSH)] for k in range(KT)]

            for j in range(KT // SW):
                qtl = deq_pool.tile([128, W], i32, tag="qtile", name="qtl")
                sctl = deq_pool.tile([128, W], f32, tag="sctile", name="sctl")
                atl = deq_pool.tile([R, SW * 128], f32, tag="atile", name="atl")
                for s in range(SW):
                    k = j * SW + s
                    nc.gpsimd.dma_start(out=qtl[:, ts(s, OSH)],
                                        in_=qt[ts(k, 128), :])
                    nc.gpsimd.dma_start(out=sctl[:, ts(s, OSH)],
                                        in_=scl[ts(k, 128), :])
                    nc.gpsimd.dma_start(out=atl[:, ts(s, 128)],
                                        in_=at[:, ts(k, 128)])

                # LoRA fold: wadd = (A @ B)[k-tiles]  (psum, exact fp32)
                wadd = wadd_pool.tile([128, W], f32, tag="wadd", name="wadd")
                for s in range(SW):
                    nc.tensor.matmul(wadd[:, ts(s, OSH)], atl[:, ts(s, 128)],
                                     b_sb[:], start=True, stop=True)

                # u = (q - 7.5) * (1/7.5)  (int32 -> f32 affine, on ACT)
                u = deq_pool.tile([128, W], f32, tag="u", name="u")
                nc.scalar.activation(u[:], qtl[:],
                                     mybir.ActivationFunctionType.Copy,
                                     bias=-1.0, scale=1.0 / 7.5)
                # Horner: acc = (((c15*u)+c14)*u + ... + c1)*u
                acc = deq_pool.tile([128, W], f32, tag="acc", name="acc")
                nc.vector.tensor_scalar_mul(acc[:], u[:], float(c[15]))
                for kk in range(14, 0, -1):
                    nc.vector.scalar_tensor_tensor(
                        acc[:], acc[:], float(c[kk]), u[:],
                        Alu.add, Alu.mult)
                # acc = (acc + c0) * absmax_expanded   (in place)
                nc.vector.scalar_tensor_tensor(
                    acc[:], acc[:], float(c[0]), sctl[:], Alu.add, Alu.mult)
                # weff = wadd*2 + acc  (reads PSUM: DVE)
                nc.vector.scalar_tensor_tensor(
                    weff_s[j][:], wadd[:], SCALING, acc[:], Alu.mult, Alu.add)

            # ---- Phase B: out[g*512+m*128 : ..., :] = x @ W_eff
            def mm_halfgroup(g, k0, k1, start, stop, psums):
                for k in range(k0, k1):
                    xg = x_pool.tile([128, TG], mm_dt, tag="xg", name="xg")
                    nc.sync.dma_start(out=xg[:],
                                      in_=xt[ts(k, 128), ts(g, TG)].bitcast(mm_dt))
                    for m in range(MPG):
                        nc.tensor.matmul(
                            psums[m][:],
                            xg[:, ts(m, 128)],
                            weff[k][:],
                            start=start and (k == k0),
                            stop=stop and (k == k1 - 1))

            # B1: first halves (k < KH) of the split groups — this work only
            # needs the early weff tiles, so it runs while dequant continues.
            partials = {}
            for g in range(NG_SPLIT):
                psums = [mm_pool.tile([128, OSH], f32, tag="mmps", name="mmps")
                         for _ in range(MPG)]
                mm_halfgroup(g, 0, KH, True, True, psums)
                for m in range(MPG):
                    pt = part_pool.tile([128, OSH], f32, tag=f"part{g}_{m}",
                                        name=f"part{g}_{m}")
                    nc.scalar.copy(pt[:], psums[m][:])
                    partials[(g, m)] = pt

            # B2: second halves of the split groups; combine with partials
            for g in range(NG_SPLIT):
                psums = [mm_pool.tile([128, OSH], f32, tag="mmps", name="mmps")
                         for _ in range(MPG)]
                mm_halfgroup(g, KH, KT, True, True, psums)
                for m in range(MPG):
                    ot = o_pool.tile([128, OSH], f32, tag="ot", name="ot")
                    nc.vector.tensor_add(ot[:], psums[m][:],
                                         partials[(g, m)][:])
                    nc.scalar.dma_start(
                        out=out[ds(g * TG + m * 128, 128), :], in_=ot[:])

            # B3: remaining groups, plain full-K accumulation
            for g in range(NG_SPLIT, NG):
                psums = [mm_pool.tile([128, OSH], f32, tag="mmps", name="mmps")
                         for _ in range(MPG)]
                mm_halfgroup(g, 0, KT, True, True, psums)
                for m in range(MPG):
                    ot = o_pool.tile([128, OSH], f32, tag="ot", name="ot")
                    nc.scalar.copy(ot[:], psums[m][:])
                    nc.scalar.dma_start(
                        out=out[ds(g * TG + m * 128, 128), :], in_=ot[:])

    nc.compile()
    _CACHE[key] = nc
    return nc


def _prepare_in_maps(x, q_idx, absmax, lora_A, lora_B):
    x = np.asarray(x, dtype=np.float32)
    q_idx = np.asarray(q_idx, dtype=np.int32)
    absmax = np.asarray(absmax, dtype=np.float32)
    lora_A = np.asarray(lora_A, dtype=np.float32)
    lora_B = np.asarray(lora_B, dtype=np.float32)

    xt = np.ascontiguousarray(x.reshape(TOK, IN).T)          # [IN, TOK]
    qt_full = q_idx.T                                        # [IN, OUT] view
    at = np.ascontiguousarray(lora_A.T)                      # [R, IN]

    in_maps = []
    for cid in range(NCORES):
        sl = slice(cid * OSH, (cid + 1) * OSH)
        scale = np.repeat(np.ascontiguousarray(absmax[sl].T), QBLOCK, axis=0)
        in_maps.append({
            "xt": xt,
            "qt": np.ascontiguousarray(qt_full[:, sl]),
            "scl": np.ascontiguousarray(scale),              # [IN, OSH]
            "at": at,
            "bsh": np.ascontiguousarray(lora_B[:, sl]),
        })
    return in_maps


def _gather(results):
    shards = [results[cid]["out"] for cid in range(NCORES)]
    full = np.concatenate(shards, axis=1)                    # [TOK, OUT]
    return full.reshape(B_, S_, OUT)


def kernel(x, q_idx, absmax, lora_A, lora_B):
    from concourse.bass_utils import run_bass_kernel_spmd

    nc = _build()
    in_maps = _prepare_in_maps(x, q_idx, absmax, lora_A, lora_B)
    res = run_bass_kernel_spmd(nc, in_maps, list(range(NCORES)))
    return _gather(res.results)


# revision 14
# speedup vs baseline: 1.0775x; 1.0775x over previous
"""Trainium2 Bass kernel for nn_LoRALinear4bit.

Computes  out = x @ dequant_nf4(q_idx, absmax).T + (x @ A) @ B * 2.0
with x [4,2048,4096] f32, q_idx [4096,4096] int32 (NF4 codes),
absmax [4096,64] f32 (per-64-block scales), A [4096,16], B [16,4096].

Strategy (column / tensor parallel over 8 NeuronCores):
  * shard out_features OUT=4096 into 8 x 512; replicate x, A.
  * per core, on device:
      - dequantize W^T shard [IN, 512] from host-transposed q codes via a
        degree-15 interpolating polynomial of the NF4 codebook (exact at the
        16 integer nodes up to ~6e-6 abs err), times the expanded absmax.
      - fold the LoRA product into the weight: W_eff = W^T*scale + 2*(A@B).
      - one big matmul: out_shard[8192, 512] = x @ W_eff, accumulated over
        IN in 32 K-tiles of 128, fp32r (full-rate fp32 PE mode).
  * host gathers the 8 column shards.

Host-side work is layout only: transposes (x.T, q.T, A.T), shard slicing,
and absmax block expansion (np.repeat).
"""

import numpy as np

# problem shape (hardcoded per contract: kernel.py must be self-contained)
B_, S_, IN, OUT = 4, 2048, 4096, 4096
TOK = B_ * S_            # 8192 tokens
NCORES = 8
OSH = OUT // NCORES      # 512 out-features per core
R = 16                   # LoRA rank
SCALING = 2.0            # alpha/r = 32/16
QBLOCK = 64              # bnb absmax blocksize

KT = IN // 128           # 32 K tiles
TG = 512                 # token group per x DMA
NG = TOK // TG           # 16 token groups
MPG = TG // 128          # 4 m-tiles per group

# bitsandbytes NF4 codebook
NF4 = np.array([
    -1.0, -0.6961928009986877, -0.5250730514526367, -0.39491748809814453,
    -0.28444138169288635, -0.18477343022823334, -0.09105003625154495, 0.0,
    0.07958029955625534, 0.16093020141124725, 0.24611230194568634,
    0.33791524171829224, 0.44070982933044434, 0.5626170039176941,
    0.6989699602127075, 1.0], dtype=np.float64)


def _poly_coeffs():
    """Coefficients of the degree-15 interpolating polynomial of NF4 in
    u = (q - 7.5)/7.5 (monomial basis, increasing order)."""
    q = np.arange(16, dtype=np.float64)
    u = (q - 7.5) / 7.5
    V = np.vander(u, 16, increasing=True)
    return np.linalg.solve(V, NF4)


_CACHE = {}


def _build(mm_dtype_name="float32r"):
    """Build + compile the per-core Bass program (identical on all cores)."""
    key = mm_dtype_name
    if key in _CACHE:
        return _CACHE[key]

    import concourse.bacc as bacc
    import concourse.tile as tile
    from concourse import mybir
    from concourse.bass import ts, ds

    f32 = mybir.dt.float32
    i32 = mybir.dt.int32
    mm_dt = getattr(mybir.dt, mm_dtype_name)
    Alu = mybir.AluOpType

    c = _poly_coeffs()

    nc = bacc.Bacc("TRN2", target_bir_lowering=False, debug=False)

    xt = nc.dram_tensor("xt", [IN, TOK], f32, kind="ExternalInput").ap()
    qt = nc.dram_tensor("qt", [IN, OSH], i32, kind="ExternalInput").ap()
    scl = nc.dram_tensor("scl", [IN, OSH], f32, kind="ExternalInput").ap()
    at = nc.dram_tensor("at", [R, IN], f32, kind="ExternalInput").ap()
    bsh = nc.dram_tensor("bsh", [R, OSH], f32, kind="ExternalInput").ap()
    out = nc.dram_tensor("out", [TOK, OSH], f32, kind="ExternalOutput").ap()

    # token groups whose contraction is split k<KH | k>=KH so their first
    # half can run while dequant is still producing the later weff tiles
    KH = KT // 2
    NG_SPLIT = 10

    with tile.TileContext(nc) as tc:
        with (
            tc.tile_pool(name="weff", bufs=1) as weff_pool,
            tc.tile_pool(name="deq", bufs=2) as deq_pool,
            tc.tile_pool(name="part", bufs=1) as part_pool,
            tc.tile_pool(name="xin", bufs=6) as x_pool,
            tc.tile_pool(name="oup", bufs=3) as o_pool,
            tc.tile_pool(name="wadd_ps", bufs=1, space="PSUM") as wadd_pool,
            tc.tile_pool(name="mm_ps", bufs=7, space="PSUM") as mm_pool,
            tc.tile_pool(name="const", bufs=1) as const_pool,
        ):
            # resident constants
            b_sb = const_pool.tile([R, OSH], f32, tag="b_sb", name="b_sb")
            nc.gpsimd.dma_start(out=b_sb[:], in_=bsh[:])

            # ---- Phase A: W_eff[k] = NF4[q^T]*scale + 2*(A@B)
            # Processed as supertiles of SW k-tiles (wide free dim amortizes
            # the per-op DVE overhead of the 16-step Horner chain).
            SW = 1
            W = SW * OSH
            weff_s = []
            for j in range(KT // SW):
                w = weff_pool.tile([128, W], mm_dt, tag=f"weff{j}",
                                   name=f"weff{j}")
                weff_s.append(w)
            weff = [weff_s[k // SW][:, ts(k % SW, OSH)] for k in range(KT)]

            for j in range(KT // SW):
                qtl = deq_pool.tile([128, W], i32, tag="qtile", name="qtl")
                sctl = deq_pool.tile([128, W], f32, tag="sctile", name="sctl")
                atl = deq_pool.tile([R, SW * 128], f32, tag="atile", name="atl")
                for s in range(SW):
                    k = j * SW + s
                    nc.gpsimd.dma_start(out=qtl[:, ts(s, OSH)],
                                        in_=qt[ts(k, 128), :])
                    nc.gpsimd.dma_start(out=sctl[:, ts(s, OSH)],
                                        in_=scl[ts(k, 128), :])
                    nc.gpsimd.dma_start(out=atl[:, ts(s, 128)],
                                        in_=at[:, ts(k, 128)])

                # LoRA fold: wadd = (A @ B)[k-tiles]  (psum, exact fp32)
                wadd = wadd_pool.tile([128, W], f32, tag="wadd", name="wadd")
                for s in range(SW):
                    nc.tensor.matmul(wadd[:, ts(s, OSH)], atl[:, ts(s, 128)],
                                     b_sb[:], start=True, stop=True)

                # u = (q - 7.5) * (1/7.5)  (int32 -> f32 affine, on ACT)
                u = deq_pool.tile([128, W], f32, tag="u", name="u")
                nc.scalar.activation(u[:], qtl[:],
                                     mybir.ActivationFunctionType.Copy,
                                     bias=-1.0, scale=1.0 / 7.5)
                # Horner: acc = (((c15*u)+c14)*u + ... + c1)*u
                acc = deq_pool.tile([128, W], f32, tag="acc", name="acc")
                nc.vector.tensor_scalar_mul(acc[:], u[:], float(c[15]))
                for kk in range(14, 0, -1):
                    nc.vector.scalar_tensor_tensor(
                        acc[:], acc[:], float(c[kk]), u[:],
                        Alu.add, Alu.mult)
                # acc = (acc + c0) * absmax_expanded   (in place)
                nc.vector.scalar_tensor_tensor(
                    acc[:], acc[:], float(c[0]), sctl[:], Alu.add, Alu.mult)
                # weff = wadd*2 + acc  (reads PSUM: DVE)
                nc.vector.scalar_tensor_tensor(
                    weff_s[j][:], wadd[:], SCALING, acc[:], Alu.mult, Alu.add)

            # ---- Phase B: out[g*512+m*128 : ..., :] = x @ W_eff
            def mm_halfgroup(g, k0, k1, start, stop, psums):
                for k in range(k0, k1):
                    xg = x_pool.tile([128, TG], mm_dt, tag="xg", name="xg")
                    nc.sync.dma_start(out=xg[:],
                                      in_=xt[ts(k, 128), ts(g, TG)].bitcast(mm_dt))
                    for m in range(MPG):
                        nc.tensor.matmul(
                            psums[m][:],
                            xg[:, ts(m, 128)],
                            weff[k][:],
                            start=start and (k == k0),
                            stop=stop and (k == k1 - 1))

            # B1: first halves (k < KH) of the split groups — this work only
            # needs the early weff tiles, so it runs while dequant continues.
            partials = {}
            for g in range(NG_SPLIT):
                psums = [mm_pool.tile([128, OSH], f32, tag="mmps", name="mmps")
                         for _ in range(MPG)]
                mm_halfgroup(g, 0, KH, True, True, psums)
                for m in range(MPG):
                    pt = part_pool.tile([128, OSH], f32, tag=f"part{g}_{m}",
                                        name=f"part{g}_{m}")
                    nc.scalar.copy(pt[:], psums[m][:])
                    partials[(g, m)] = pt

            # B2: second halves of the split groups; combine with partials
            for g in range(NG_SPLIT):
                psums = [mm_pool.tile([128, OSH], f32, tag="mmps", name="mmps")
                         for _ in range(MPG)]
                mm_halfgroup(g, KH, KT, True, True, psums)
                for m in range(MPG):
                    ot = o_pool.tile([128, OSH], f32, tag="ot", name="ot")
                    nc.vector.tensor_add(ot[:], psums[m][:],
                                         partials[(g, m)][:])
                    nc.scalar.dma_start(
                        out=out[ds(g * TG + m * 128, 128), :], in_=ot[:])

            # B3: remaining groups, plain full-K accumulation
            for g in range(NG_SPLIT, NG):
                psums = [mm_pool.tile([128, OSH], f32, tag="mmps", name="mmps")
                         for _ in range(MPG)]
                mm_halfgroup(g, 0, KT, True, True, psums)
                for m in range(MPG):
                    ot = o_pool.tile([128, OSH], f32, tag="ot", name="ot")
                    nc.scalar.copy(ot[:], psums[m][:])
                    nc.scalar.dma_start(
                        out=out[ds(g * TG + m * 128, 128), :], in_=ot[:])

    nc.compile()
    _CACHE[key] = nc
    return nc


def _prepare_in_maps(x, q_idx, absmax, lora_A, lora_B):
    x = np.asarray(x, dtype=np.float32)
    q_idx = np.asarray(q_idx, dtype=np.int32)
    absmax = np.asarray(absmax, dtype=np.float32)
    lora_A = np.asarray(lora_A, dtype=np.float32)
    lora_B = np.asarray(lora_B, dtype=np.float32)

    xt = np.ascontiguousarray(x.reshape(TOK, IN).T)          # [IN, TOK]
    qt_full = q_idx.T                                        # [IN, OUT] view
    at = np.ascontiguousarray(lora_A.T)                      # [R, IN]

    in_maps = []
    for cid in range(NCORES):
        sl = slice(cid * OSH, (cid + 1) * OSH)
        scale = np.repeat(np.ascontiguousarray(absmax[sl].T), QBLOCK, axis=0)
        in_maps.append({
            "xt": xt,
            "qt": np.ascontiguousarray(qt_full[:, sl]),
            "scl": np.ascontiguousarray(scale),              # [IN, OSH]
            "at": at,
            "bsh": np.ascontiguousarray(lora_B[:, sl]),
        })
    return in_maps


def _gather(results):
    shards = [results[cid]["out"] for cid in range(NCORES)]
    full = np.concatenate(shards, axis=1)                    # [TOK, OUT]
    return full.reshape(B_, S_, OUT)


def kernel(x, q_idx, absmax, lora_A, lora_B):
    from concourse.bass_utils import run_bass_kernel_spmd

    nc = _build()
    in_maps = _prepare_in_maps(x, q_idx, absmax, lora_A, lora_B)
    res = run_bass_kernel_spmd(nc, in_maps, list(range(NCORES)))
    return _gather(res.results)
